# revision 1
# baseline (speedup 1.0000x reference)
"""Depthwise 4x4 blur (upfirdn2d pad=(2,1)) on TRN2, 8 NeuronCores.

Math: out[h,w] = sum_{i,j} Kf[i,j] * x[h+i-2, w+j-2]   (Kf = flipped 2D kernel,
out-of-range terms = zero padding). For each kernel column j this is a banded
128x128 matrix A_j applied over H to a W-shifted slice of the padded image:

    OUT = sum_j A_j @ Xpad[:, j:j+128]      (PSUM accumulation over j)

so one image needs 4 TensorE matmuls and no transposes. H-padding is folded
into the band clipping of A_j; W-padding is baked into the host-side layout
(stride-131 rows: [0, 0, x0..x127, 0]). Sharding: batch dim (8 batches ->
8 cores), each core processes 256 images of 128x128.

float32r facts (measured on TRN2 HW):
  - matmul operands tagged float32r stream the PE at 1 col/cycle (float32: 4).
  - the PE is bit-exact when operands have <= 11 mantissa bits; full-mantissa
    operands behave as if rounded (rel err ~1.3e-4).
  - DVE tensor_copy into a float32r tile rounds to 11 mantissa bits (RNE);
    DVE subtract with float32r output then gives an exact lo = x - hi.
So mode "hilo" splits x = hi + lo on-chip and accumulates all 8 band matmuls
(4 shifts x {hi, lo}) into the same PSUM tile: full fp32 accuracy at the fast
PE rate. Mode "f32r" (4 matmuls) is ~15% faster with ~1.5e-4 rel err.
"""

import numpy as np
from contextlib import ExitStack

import concourse.bass as bass
import concourse.bacc as bacc
import concourse.tile as tile
import concourse.mybir as mybir
from concourse.bass_utils import run_bass_kernel_spmd

N_CORES = 8
B, C, H, W = 8, 256, 128, 128
WP = W + 3         # padded image stride: [0, 0, x0..x127, 0]
GROUP = 4          # images per PSUM bank (4*128 = 512 f32 = one bank)
SUPER = 16         # images per DMA (~1 MB transfers)
MODE = "hilo"      # "hilo" (fp32-exact) | "f32r" (fast, ~1.5e-4) | "f32" (slow exact)

F32 = mybir.dt.float32
F32R = mybir.dt.float32r


def _body(ctx, tc, o_ap, x_ap, w_ap, mode, ramp=True, pair=True):
    nc = tc.nc
    mm_dt = F32 if mode == "f32" else F32R
    wpool = ctx.enter_context(tc.tile_pool(name="wts", bufs=1))
    xpool = ctx.enter_context(tc.tile_pool(name="xin", bufs=4))
    opool = ctx.enter_context(tc.tile_pool(name="oup", bufs=4))
    ppool = ctx.enter_context(tc.tile_pool(name="ps", bufs=8, space="PSUM"))
    if mode == "hilo":
        hpool = ctx.enter_context(tc.tile_pool(name="xhi", bufs=4))
        lpool = ctx.enter_context(tc.tile_pool(name="xlo", bufs=4))

    # weights arrive host-pre-arranged as [H, 4*H] (k-major, contiguous rows:
    # one 2KB descriptor per partition) on the ACT ring so the first data
    # tile leads the SP ring
    wt = wpool.tile([H, 4 * H], mm_dt)
    nc.scalar.dma_start(wt[:], w_ap)

    # ramp-up / ramp-down supertile sizes: small tiles at the ends prime and
    # drain the DMA->split->matmul->copy->DMA pipeline faster
    if ramp:
        sizes = [2, 2, 4, 8] + [SUPER] * ((C - 32) // SUPER) + [8, 4, 2, 2]
    else:
        sizes = [SUPER] * (C // SUPER)
    assert sum(sizes) == C
    c0 = 0
    for sz in sizes:
        xt = xpool.tile([H, sz * WP], F32 if mode == "hilo" else mm_dt, tag="xt")
        xt3 = xt[:].rearrange("h (c w) -> h c w", c=sz)
        if 2 <= sz <= 8:
            # small ramp tiles: split across both HWDGE rings so descriptor
            # generation for the two halves runs in parallel
            hh = sz // 2
            nc.sync.dma_start(
                xt3[:, :hh], x_ap[c0 : c0 + hh].rearrange("c h w -> h c w")
            )
            nc.scalar.dma_start(
                xt3[:, hh:], x_ap[c0 + hh : c0 + sz].rearrange("c h w -> h c w")
            )
        else:
            nc.sync.dma_start(xt3, x_ap[c0 : c0 + sz].rearrange("c h w -> h c w"))
        if mode == "hilo":
            xhi = hpool.tile([H, sz * WP], F32R, tag="xhi")
            xlo = lpool.tile([H, sz * WP], F32R, tag="xlo")
            parts = [
                xhi[:].rearrange("h (c w) -> h c w", c=sz),
                xlo[:].rearrange("h (c w) -> h c w", c=sz),
            ]
        else:
            parts = [xt3]
        ot = opool.tile([H, sz * W], F32, tag="ot")
        # PSUM groups: (img_start, img_count) within the supertile; paired so
        # consecutive matmuls reuse each stationary weight
        if sz >= GROUP:
            groups = [(i * GROUP, GROUP) for i in range(sz // GROUP)]
        else:
            groups = [(0, sz)]
        if pair:
            pairs = [tuple(groups[i : i + 2]) for i in range(0, len(groups), 2)]
        else:
            pairs = [(g,) for g in groups]
        for gs in pairs:
            i0, iend = gs[0][0], gs[-1][0] + gs[-1][1]
            if mode == "hilo":
                for gi, gc in gs:
                    cs = slice(gi * WP, (gi + gc) * WP)
                    nc.vector.tensor_copy(xhi[:, cs], xt[:, cs])  # RNE, 11 bits
                    nc.vector.tensor_tensor(
                        xlo[:, cs], xt[:, cs], xhi[:, cs].bitcast(F32),
                        mybir.AluOpType.subtract,
                    )
            pts = []
            for g in gs:
                pt = ppool.tile([H, g[1] * W], F32, tag="pt")
                pts.append(pt)
            n_mm = 4 * len(parts)
            k = 0
            for j in range(4):
                lhsT = wt[:, j * H : (j + 1) * H]
                for p3 in parts:
                    for (gi, gc), pt in zip(gs, pts):
                        rhs = p3[:, gi : gi + gc, j : j + W]
                        nc.tensor.matmul(
                            pt[:], lhsT, rhs, start=(k == 0), stop=(k == n_mm - 1)
                        )
                    k += 1
            for (gi, gc), pt in zip(gs, pts):
                nc.scalar.copy(ot[:, gi * W : (gi + gc) * W], pt[:])
            # per-pair output DMA, alternating rings, to drain early and thin
            # the kernel tail
            eng = nc.sync if (c0 + i0) % 8 else nc.scalar
            eng.dma_start(
                o_ap[c0 + i0 : c0 + iend].rearrange("c h w -> h c w"),
                ot[:, i0 * W : iend * W].rearrange(
                    "h (c w) -> h c w", c=iend - i0
                ),
            )
        c0 += sz


def build_module(mode=MODE, ramp=True, pair=True):
    nc = bacc.Bacc(
        "TRN2", target_bir_lowering=False, debug=False, num_devices=N_CORES
    )
    x_dt = F32 if mode in ("hilo", "f32") else F32R
    w_dt = F32 if mode == "f32" else F32R
    x_ap = nc.dram_tensor("x", [C, H, WP], x_dt, kind="ExternalInput").ap()
    w_ap = nc.dram_tensor("wts", [H, 4 * H], w_dt, kind="ExternalInput").ap()
    o_ap = nc.dram_tensor("out", [C, H, W], F32, kind="ExternalOutput").ap()
    with tile.TileContext(nc) as tc:
        with ExitStack() as ctx:
            _body(ctx, tc, o_ap, x_ap, w_ap, mode, ramp=ramp, pair=pair)
    nc.compile()
    return nc


def band_mats(k2d):
    """WT[j] = A_j^T where A_j[h, h+i-2] = Kf[i, j] (rows clipped to [0,128))."""
    kf = np.asarray(k2d, np.float32)[::-1, ::-1]
    wts = np.zeros((4, H, H), np.float32)
    for j in range(4):
        for i in range(4):
            d = i - 2  # diagonal offset m - h
            h0, h1 = max(0, -d), min(H, H - d)
            idx = np.arange(h0, h1)
            wts[j, idx + d, idx] = kf[i, j]
    return wts


def pad_w(x_core):
    """[C,H,W] f32 -> [C,H,WP] with zero cols at 0,1 and WP-1."""
    xp = np.zeros((x_core.shape[0], H, WP), np.float32)
    xp[:, :, 2 : 2 + W] = x_core
    return xp


_module_cache = {}


def _get_module(mode=MODE):
    if mode not in _module_cache:
        _module_cache[mode] = build_module(mode)
    return _module_cache[mode]


def kernel(x, kernel, _trace=False, _trace_kwargs=None, _mode=None):
    x = np.asarray(x, np.float32)
    assert x.shape == (B, C, H, W), x.shape
    wts = band_mats(kernel).transpose(1, 0, 2).reshape(H, 4 * H).copy()
    nc = _get_module(_mode or MODE)
    in_maps = [{"x": pad_w(x[i]), "wts": wts} for i in range(N_CORES)]
    res = run_bass_kernel_spmd(
        nc, in_maps, list(range(N_CORES)), trace=_trace, **(_trace_kwargs or {})
    )
    out = np.stack([res.results[i]["out"] for i in range(N_CORES)], axis=0)
    if _trace:
        return out, res
    return out



# revision 2
# speedup vs baseline: 2.0676x; 2.0676x over previous
"""Depthwise 4x4 blur (upfirdn2d pad=(2,1)) on TRN2, 8 NeuronCores.

Math: out[h,w] = sum_{i,j} Kf[i,j] * x[h+i-2, w+j-2]   (Kf = flipped 2D kernel,
out-of-range terms = zero padding). For each kernel column j this is a banded
128x128 matrix A_j applied over H to a W-shifted slice of the padded image:

    OUT = sum_j A_j @ Xpad[:, j:j+128]      (PSUM accumulation over j)

so one image needs 4 TensorE matmuls and no transposes. H-padding is folded
into the band clipping of A_j; W-padding is baked into the host-side layout
(stride-131 rows: [0, 0, x0..x127, 0]). Sharding: batch dim (8 batches ->
8 cores), each core processes 256 images of 128x128.

v2 (bf16): the harness tolerance is 2e-2, so bf16 operands are ample
(measured rel err ~1e-3). This
  - halves HBM traffic vs f32 (in 8.2 MiB + out 8 MiB per core), and
  - streams the PE at 1 col/cycle without the hilo split (4 matmuls per
    image group instead of 8).
Host-side the image block is transposed to [H, C, WP] (and the output comes
back as [H, C, W]), so every DMA is a dense 2D pattern: each partition h
reads/writes ONE contiguous multi-KB run per supertile instead of per-image
~500 B strided chunks (the v1 trace showed ~66k descriptors at ~510 B -> all
16 SDMA queues descriptor-overhead-bound). PSUM->SBUF copies ride the Vector
engine, which has nothing else to do in bf16 mode.
"""

import numpy as np
from contextlib import ExitStack

import concourse.bass as bass
import concourse.bacc as bacc
import concourse.tile as tile
import concourse.mybir as mybir
from concourse.bass_utils import run_bass_kernel_spmd

N_CORES = 8
B, C, H, W = 8, 256, 128, 128
WP = W + 3         # padded image stride: [0, 0, x0..x127, 0]
GROUP = 4          # images per PSUM bank (4*128 = 512 f32 = one bank)
SUPER = 16         # images per DMA

F32 = mybir.dt.float32
BF16 = mybir.dt.bfloat16


def _body(ctx, tc, o_ap, x_ap, w_ap, ramp=True):
    nc = tc.nc
    wpool = ctx.enter_context(tc.tile_pool(name="wts", bufs=1))
    xpool = ctx.enter_context(tc.tile_pool(name="xin", bufs=4))
    opool = ctx.enter_context(tc.tile_pool(name="oup", bufs=4))
    ppool = ctx.enter_context(tc.tile_pool(name="ps", bufs=8, space="PSUM"))

    # weights arrive host-pre-arranged as [H, 4*H] (k-major) on the ACT ring
    # so the first data tile leads the SP ring
    wt = wpool.tile([H, 4 * H], BF16)
    nc.scalar.dma_start(wt[:], w_ap)

    # ramp-up / ramp-down supertile sizes: small tiles at the ends prime and
    # drain the DMA->matmul->copy->DMA pipeline faster
    if ramp:
        n_full = (C - 32) // SUPER
        sizes = [4, 4, 8] + [SUPER] * n_full + [8, 4, 4]
    else:
        sizes = [SUPER] * (C // SUPER)
    assert sum(sizes) == C
    c0 = 0
    for sz in sizes:
        xt = xpool.tile([H, sz * WP], BF16, tag="xt")
        xt3 = xt[:].rearrange("h (c w) -> h c w", c=sz)
        nc.sync.dma_start(xt3, x_ap[:, c0 : c0 + sz])
        ot = opool.tile([H, sz * W], BF16, tag="ot")
        groups = [(i * GROUP, min(GROUP, sz - i * GROUP))
                  for i in range((sz + GROUP - 1) // GROUP)]
        for gi, gc in groups:
            pt = ppool.tile([H, gc * W], F32, tag="pt")
            for j in range(4):
                lhsT = wt[:, j * H : (j + 1) * H]
                rhs = xt3[:, gi : gi + gc, j : j + W]
                nc.tensor.matmul(pt[:], lhsT, rhs, start=(j == 0), stop=(j == 3))
            nc.vector.tensor_copy(ot[:, gi * W : (gi + gc) * W], pt[:])
        # one output DMA per supertile on the ACT ring (input owns SP ring)
        nc.scalar.dma_start(
            o_ap[:, c0 : c0 + sz],
            ot[:].rearrange("h (c w) -> h c w", c=sz),
        )
        c0 += sz


def build_module(ramp=True):
    nc = bacc.Bacc(
        "TRN2", target_bir_lowering=False, debug=False, num_devices=N_CORES
    )
    x_ap = nc.dram_tensor("x", [H, C, WP], BF16, kind="ExternalInput").ap()
    w_ap = nc.dram_tensor("wts", [H, 4 * H], BF16, kind="ExternalInput").ap()
    o_ap = nc.dram_tensor("out", [H, C, W], BF16, kind="ExternalOutput").ap()
    with tile.TileContext(nc) as tc:
        with ExitStack() as ctx:
            _body(ctx, tc, o_ap, x_ap, w_ap, ramp=ramp)
    nc.compile()
    return nc


def band_mats(k2d):
    """WT[j] = A_j^T where A_j[h, h+i-2] = Kf[i, j] (rows clipped to [0,128))."""
    kf = np.asarray(k2d, np.float32)[::-1, ::-1]
    wts = np.zeros((4, H, H), np.float32)
    for j in range(4):
        for i in range(4):
            d = i - 2  # diagonal offset m - h
            h0, h1 = max(0, -d), min(H, H - d)
            idx = np.arange(h0, h1)
            wts[j, idx + d, idx] = kf[i, j]
    return wts


def _bf16(a):
    import ml_dtypes

    return np.asarray(a).astype(ml_dtypes.bfloat16)


def prep_x(x_core):
    """[C,H,W] f32 -> [H,C,WP] bf16 with zero cols at 0,1 and WP-1."""
    xp = np.zeros((H, x_core.shape[0], WP), np.float32)
    xp[:, :, 2 : 2 + W] = x_core.transpose(1, 0, 2)
    return _bf16(xp)


_module_cache = {}


def _get_module():
    if "m" not in _module_cache:
        _module_cache["m"] = build_module()
    return _module_cache["m"]


def kernel(x, kernel, _trace=False, _trace_kwargs=None):
    x = np.asarray(x, np.float32)
    assert x.shape == (B, C, H, W), x.shape
    wts = _bf16(band_mats(kernel).transpose(1, 0, 2).reshape(H, 4 * H))
    nc = _get_module()
    in_maps = [{"x": prep_x(x[i]), "wts": wts.copy()} for i in range(N_CORES)]
    res = run_bass_kernel_spmd(
        nc, in_maps, list(range(N_CORES)), trace=_trace, **(_trace_kwargs or {})
    )
    out = np.stack(
        [
            np.asarray(res.results[i]["out"]).transpose(1, 0, 2).astype(np.float32)
            for i in range(N_CORES)
        ],
        axis=0,
    )
    if _trace:
        return out, res
    return out
